# revision 1
# baseline (speedup 1.0000x reference)
"""Multi-head attention (4 heads, N=8192, F=64, KQ=64, HV=16) on 8 trn2 cores.

Sharding: core c owns head h = c//2 and query half qh = c%2 (4096 rows).
Each core runs a flash-attention-style kernel entirely on-chip:
  - K^T, Q^T projections on PE (contraction over F=64), biases added on DVE.
  - V' chunks [128, 17] with a ones column (folds the softmax denominator
    into the attention@V matmul).
  - Steady state per 512-query block: PE computes S^T tiles [128k, 512q]
    into PSUM, ACT applies exp(0.125*s) into SBUF, PE accumulates
    attn_unnorm^T [17, 512] over all 64 k-chunks.
  - Epilogue: PE-transpose back to [q, d], DVE reciprocal+scale, DMA out.

The softmax max-subtraction is skipped: logits are bounded (|s|*0.125 < ~2
for this problem's scale-0.05 weights), exp is safe in fp32.

Everything is fp32. The per-core programs are identical (pure data SPMD);
per-core differences live in the input tensors only.
"""

import os
import sys

import numpy as np

if "/opt/trn_rl_repo" not in sys.path:
    sys.path.insert(0, "/opt/trn_rl_repo")

N = 8192
F = 64
KQ = 64
HV = 16
NH = 4
NQ = N // 2          # query rows per core
QB = 512             # query block width
NBLK = NQ // QB      # 8 query blocks per core
KCH = N // 128       # 64 k-chunks of 128
GRP = 3              # k-chunks per exp group (3 PSUM banks)
SCALE = 1.0 / 8.0    # 1/sqrt(F)

_CACHE = {}

LAST_EXEC_NS = None


def _build_program():
    import concourse.bacc as bacc
    import concourse.mybir as mybir
    import concourse.tile as tile
    from concourse.masks import make_identity

    fp32 = mybir.dt.float32
    Exp = mybir.ActivationFunctionType.Exp
    Add = mybir.AluOpType.add

    nc = bacc.Bacc("TRN2", target_bir_lowering=False, debug=False, num_devices=8)

    xfT_d = nc.declare_dram_parameter("xfT", [F, N], fp32, isOutput=False)
    xqT_d = nc.declare_dram_parameter("xqT", [F, NQ], fp32, isOutput=False)
    wq_d = nc.declare_dram_parameter("wq", [F, KQ], fp32, isOutput=False)
    bq_d = nc.declare_dram_parameter("bq", [KQ, 1], fp32, isOutput=False)
    wk_d = nc.declare_dram_parameter("wk", [F, KQ], fp32, isOutput=False)
    bk_d = nc.declare_dram_parameter("bk", [KQ, 1], fp32, isOutput=False)
    wv_d = nc.declare_dram_parameter("wv", [F + 1, HV + 1], fp32, isOutput=False)
    out_d = nc.declare_dram_parameter("out", [NQ, HV], fp32, isOutput=True)

    with tile.TileContext(nc) as tc:
        with (
            tc.tile_pool(name="persist", bufs=1) as pp,
            tc.tile_pool(name="expp", bufs=3) as expp,
            tc.tile_pool(name="accsp", bufs=2) as accsp,
            tc.tile_pool(name="recp", bufs=2) as recp,
            tc.tile_pool(name="outp", bufs=2) as outp,
            tc.tile_pool(name="stp", bufs=2, space="PSUM") as stp,
            tc.tile_pool(name="accp", bufs=2, space="PSUM") as accp,
        ):
            # ---- persistent SBUF tensors ----
            xft = pp.tile([F + 1, N], fp32, tag="xft")      # x^T plus ones row
            xqt = pp.tile([F, NQ], fp32, tag="xqt")
            kt = pp.tile([KQ, N], fp32, tag="kt")           # K^T (with bias)
            qt = pp.tile([KQ, NQ], fp32, tag="qt")          # Q^T (with bias)
            vp = pp.tile([128, KCH * (HV + 1)], fp32, tag="vp")  # V' chunks
            wq = pp.tile([F, KQ], fp32, tag="wq")
            wk = pp.tile([F, KQ], fp32, tag="wk")
            wv = pp.tile([F + 1, HV + 1], fp32, tag="wv")
            bq = pp.tile([KQ, 1], fp32, tag="bq")
            bk = pp.tile([KQ, 1], fp32, tag="bk")
            ident = pp.tile([128, 128], fp32, tag="ident")

            # ---- input DMAs ----
            nc.sync.dma_start(out=xft[0:F, :], in_=xfT_d[:])
            nc.gpsimd.memset(xft[F : F + 1, :], 1.0)
            nc.sync.dma_start(out=xqt[:], in_=xqT_d[:])
            nc.sync.dma_start(out=wq[:], in_=wq_d[:])
            nc.sync.dma_start(out=wk[:], in_=wk_d[:])
            nc.sync.dma_start(out=wv[:], in_=wv_d[:])
            nc.sync.dma_start(out=bq[:], in_=bq_d[:])
            nc.sync.dma_start(out=bk[:], in_=bk_d[:])
            make_identity(nc, ident[:])

            # ---- projections: K^T over all N, Q^T over this core's half ----
            def project(dst, w_t, b_t, src, n_cols):
                nch = n_cols // QB
                g = 0
                while g * GRP < nch:
                    lo = g * GRP
                    hi = min(lo + GRP, nch)
                    ptile = stp.tile([128, GRP * QB], fp32, tag="st")
                    for i in range(hi - lo):
                        c = lo + i
                        nc.tensor.matmul(
                            ptile[0:KQ, QB * i : QB * (i + 1)],
                            lhsT=w_t[:],
                            rhs=src[0:F, QB * c : QB * (c + 1)],
                            start=True,
                            stop=True,
                        )
                    w = QB * (hi - lo)
                    nc.vector.tensor_tensor(
                        dst[:, QB * lo : QB * lo + w],
                        ptile[0:KQ, 0:w],
                        b_t[:].to_broadcast([KQ, w]),
                        Add,
                    )
                    g += 1

            project(kt, wk, bk, xft, N)
            project(qt, wq, bq, xqt, NQ)

            # ---- V' chunks: [128, 17] each, 8 per PSUM staging tile ----
            for vg in range(KCH // 8):
                vt = accp.tile([128, QB], fp32, tag="acc")
                for i in range(8):
                    j = 8 * vg + i
                    nc.tensor.matmul(
                        vt[:, (HV + 1) * i : (HV + 1) * (i + 1)],
                        lhsT=xft[:, 128 * j : 128 * (j + 1)],
                        rhs=wv[:],
                        start=True,
                        stop=True,
                    )
                wv_w = 8 * (HV + 1)
                nc.vector.tensor_copy(
                    out=vp[:, wv_w * vg : wv_w * (vg + 1)], in_=vt[:, 0:wv_w]
                )

            # ---- steady state ----
            n_grp = (KCH + GRP - 1) // GRP
            for b in range(NBLK):
                acc = accp.tile([128, QB], fp32, tag="acc")
                q_rhs = qt[:, QB * b : QB * (b + 1)]
                for g in range(n_grp):
                    lo = g * GRP
                    hi = min(lo + GRP, KCH)
                    st = stp.tile([128, GRP * QB], fp32, tag="st")
                    ex = expp.tile([128, GRP * QB], fp32, tag="ex")
                    for i in range(hi - lo):
                        j = lo + i
                        nc.tensor.matmul(
                            st[:, QB * i : QB * (i + 1)],
                            lhsT=kt[:, 128 * j : 128 * (j + 1)],
                            rhs=q_rhs,
                            start=True,
                            stop=True,
                        )
                    w = QB * (hi - lo)
                    nc.scalar.activation(
                        out=ex[:, 0:w], in_=st[:, 0:w], func=Exp, scale=SCALE
                    )
                    for i in range(hi - lo):
                        j = lo + i
                        nc.tensor.matmul(
                            acc[0 : HV + 1, :],
                            lhsT=vp[:, (HV + 1) * j : (HV + 1) * (j + 1)],
                            rhs=ex[:, QB * i : QB * (i + 1)],
                            start=(j == 0),
                            stop=(j == KCH - 1),
                        )

                # ---- block epilogue ----
                accs = accsp.tile([HV + 1, QB], fp32, tag="accs")
                nc.vector.tensor_copy(out=accs[:], in_=acc[0 : HV + 1, :])
                tp = accp.tile([128, QB], fp32, tag="acc")
                for t in range(QB // 128):
                    nc.tensor.transpose(
                        tp[:, (HV + 1) * t : (HV + 1) * (t + 1)],
                        accs[:, 128 * t : 128 * (t + 1)],
                        ident[0 : HV + 1, 0 : HV + 1],
                    )
                rec = recp.tile([128, QB // 128], fp32, tag="rec")
                sums = tp[:, 0 : 4 * (HV + 1)].rearrange(
                    "p (t c) -> p t c", c=HV + 1
                )[:, :, HV]
                nc.vector.reciprocal(rec[:], sums)
                outt = outp.tile([128, 4 * HV], fp32, tag="outt")
                for t in range(QB // 128):
                    nc.vector.tensor_scalar_mul(
                        outt[:, HV * t : HV * (t + 1)],
                        tp[:, (HV + 1) * t : (HV + 1) * t + HV],
                        rec[:, t : t + 1],
                    )
                dview = out_d[QB * b : QB * (b + 1), :].rearrange(
                    "(t p) d -> p t d", p=128
                )
                with nc.allow_non_contiguous_dma(reason="64B-run output scatter"):
                    nc.sync.dma_start(
                        out=dview, in_=outt[:].rearrange("p (t d) -> p t d", d=HV)
                    )

    nc.compile()
    return nc


def _get_program():
    if "nc" not in _CACHE:
        _CACHE["nc"] = _build_program()
    return _CACHE["nc"]


def kernel(x, Wq, bq, Wk, bk, Wv, bv):
    global LAST_EXEC_NS
    from concourse.bass_utils import run_bass_kernel_spmd

    x = np.ascontiguousarray(np.asarray(x, dtype=np.float32))
    Wq = np.asarray(Wq, dtype=np.float32)
    bq = np.asarray(bq, dtype=np.float32)
    Wk = np.asarray(Wk, dtype=np.float32)
    bk = np.asarray(bk, dtype=np.float32)
    Wv = np.asarray(Wv, dtype=np.float32)
    bv = np.asarray(bv, dtype=np.float32)

    xfT = np.ascontiguousarray(x.T)                      # [64, 8192]
    xqT = [np.ascontiguousarray(x[i * NQ : (i + 1) * NQ].T) for i in range(2)]

    in_maps = []
    for c in range(8):
        h, qh = c // 2, c % 2
        wv_aug = np.zeros((F + 1, HV + 1), dtype=np.float32)
        wv_aug[0:F, 0:HV] = Wv[h]
        wv_aug[F, 0:HV] = bv[h]
        wv_aug[F, HV] = 1.0
        in_maps.append(
            {
                "xfT": xfT,
                "xqT": xqT[qh],
                "wq": np.ascontiguousarray(Wq[h]),
                "bq": np.ascontiguousarray(bq[h].reshape(KQ, 1)),
                "wk": np.ascontiguousarray(Wk[h]),
                "bk": np.ascontiguousarray(bk[h].reshape(KQ, 1)),
                "wv": wv_aug,
            }
        )

    nc = _get_program()
    trace = bool(os.environ.get("BASS_KERNEL_TRACE"))
    res = run_bass_kernel_spmd(nc, in_maps, list(range(8)), trace=trace)
    LAST_EXEC_NS = res.exec_time_ns

    out = np.empty((N, NH * HV), dtype=np.float32)
    for c in range(8):
        h, qh = c // 2, c % 2
        out[qh * NQ : (qh + 1) * NQ, h * HV : (h + 1) * HV] = res.results[c]["out"]
    return out


# revision 2
# speedup vs baseline: 2.9308x; 2.9308x over previous
"""Multi-head attention (4 heads, N=8192, F=64, KQ=64, HV=16) on 8 trn2 cores.

Sharding: core c owns head h = c//2 and query half qh = c%2 (4096 rows).
Each core runs a flash-attention-style kernel entirely on-chip:
  - K^T, Q^T projections on PE (contraction over F=64), biases added on DVE.
  - V' chunks [128, 17] with a ones column (folds the softmax denominator
    into the attention@V matmul).
  - Steady state per 512-query block: PE computes S^T tiles [128k, 512q]
    into PSUM, ACT applies exp(0.125*s) into SBUF, PE accumulates
    attn_unnorm^T [17, 512] over all 64 k-chunks.
  - Epilogue: PE-transpose back to [q, d], DVE reciprocal+scale, DMA out.

The softmax max-subtraction is skipped: logits are bounded (|s|*0.125 < ~2
for this problem's scale-0.05 weights), exp is safe in fp32.

Everything is fp32. The per-core programs are identical (pure data SPMD);
per-core differences live in the input tensors only.
"""

import os
import sys

import numpy as np

if "/opt/trn_rl_repo" not in sys.path:
    sys.path.insert(0, "/opt/trn_rl_repo")

N = 8192
F = 64
KQ = 64
HV = 16
NH = 4
NQ = N // 2          # query rows per core
QB = 512             # query block width
NBLK = NQ // QB      # 8 query blocks per core
KCH = N // 128       # 64 k-chunks of 128
GRP = 3              # k-chunks per exp group (3 PSUM banks)
SCALE = 1.0 / 8.0    # 1/sqrt(F)

_CACHE = {}

LAST_EXEC_NS = None


def _build_program():
    import concourse.bacc as bacc
    import concourse.mybir as mybir
    import concourse.tile as tile
    from concourse.masks import make_identity

    fp32 = mybir.dt.float32
    f32r = mybir.dt.float32r
    Exp = mybir.ActivationFunctionType.Exp
    Add = mybir.AluOpType.add

    nc = bacc.Bacc("TRN2", target_bir_lowering=False, debug=False, num_devices=8)

    xfT_d = nc.declare_dram_parameter("xfT", [F, N], fp32, isOutput=False)
    xqT_d = nc.declare_dram_parameter("xqT", [F, NQ], fp32, isOutput=False)
    wq_d = nc.declare_dram_parameter("wq", [F, KQ], fp32, isOutput=False)
    bq_d = nc.declare_dram_parameter("bq", [KQ, 1], fp32, isOutput=False)
    wk_d = nc.declare_dram_parameter("wk", [F, KQ], fp32, isOutput=False)
    bk_d = nc.declare_dram_parameter("bk", [KQ, 1], fp32, isOutput=False)
    wv_d = nc.declare_dram_parameter("wv", [F + 1, HV + 1], fp32, isOutput=False)
    out_d = nc.declare_dram_parameter("out", [NQ, HV], fp32, isOutput=True)

    with tile.TileContext(nc) as tc:
        with (
            tc.tile_pool(name="persist", bufs=1) as pp,
            tc.tile_pool(name="expp", bufs=3) as expp,
            tc.tile_pool(name="accsp", bufs=2) as accsp,
            tc.tile_pool(name="recp", bufs=2) as recp,
            tc.tile_pool(name="outp", bufs=2) as outp,
            tc.tile_pool(name="stp", bufs=2, space="PSUM") as stp,
            tc.tile_pool(name="accp", bufs=2, space="PSUM") as accp,
        ):
            # ---- persistent SBUF tensors ----
            xft = pp.tile([F + 1, N], fp32, tag="xft")      # x^T plus ones row
            xqt = pp.tile([F, NQ], fp32, tag="xqt")
            kt = pp.tile([KQ, N], f32r, tag="kt")           # K^T (with bias)
            qt = pp.tile([KQ, NQ], f32r, tag="qt")          # Q^T (with bias)
            vp = pp.tile([128, KCH * (HV + 1)], f32r, tag="vp")  # V' chunks
            wq = pp.tile([F, KQ], fp32, tag="wq")
            wk = pp.tile([F, KQ], fp32, tag="wk")
            wv = pp.tile([F + 1, HV + 1], fp32, tag="wv")
            bq = pp.tile([KQ, 1], fp32, tag="bq")
            bk = pp.tile([KQ, 1], fp32, tag="bk")
            ident = pp.tile([128, 128], fp32, tag="ident")

            # ---- input DMAs ----
            nc.sync.dma_start(out=xft[0:F, :], in_=xfT_d[:])
            nc.gpsimd.memset(xft[F : F + 1, :], 1.0)
            nc.sync.dma_start(out=xqt[:], in_=xqT_d[:])
            nc.sync.dma_start(out=wq[:], in_=wq_d[:])
            nc.sync.dma_start(out=wk[:], in_=wk_d[:])
            nc.sync.dma_start(out=wv[:], in_=wv_d[:])
            nc.sync.dma_start(out=bq[:], in_=bq_d[:])
            nc.sync.dma_start(out=bk[:], in_=bk_d[:])
            make_identity(nc, ident[:])

            # ---- projections: K^T over all N, Q^T over this core's half ----
            def project(dst, w_t, b_t, src, n_cols):
                nch = n_cols // QB
                g = 0
                while g * GRP < nch:
                    lo = g * GRP
                    hi = min(lo + GRP, nch)
                    ptile = stp.tile([128, GRP * QB], fp32, tag="st")
                    for i in range(hi - lo):
                        c = lo + i
                        nc.tensor.matmul(
                            ptile[0:KQ, QB * i : QB * (i + 1)],
                            lhsT=w_t[:],
                            rhs=src[0:F, QB * c : QB * (c + 1)],
                            start=True,
                            stop=True,
                        )
                    w = QB * (hi - lo)
                    nc.vector.tensor_tensor(
                        dst[:, QB * lo : QB * lo + w],
                        ptile[0:KQ, 0:w],
                        b_t[:].to_broadcast([KQ, w]),
                        Add,
                    )
                    g += 1

            project(kt, wk, bk, xft, N)
            project(qt, wq, bq, xqt, NQ)

            # ---- V' chunks: [128, 17] each, 8 per PSUM staging tile ----
            for vg in range(KCH // 8):
                vt = accp.tile([128, QB], fp32, tag="acc")
                for i in range(8):
                    j = 8 * vg + i
                    nc.tensor.matmul(
                        vt[:, (HV + 1) * i : (HV + 1) * (i + 1)],
                        lhsT=xft[:, 128 * j : 128 * (j + 1)],
                        rhs=wv[:],
                        start=True,
                        stop=True,
                    )
                wv_w = 8 * (HV + 1)
                nc.vector.tensor_copy(
                    out=vp[:, wv_w * vg : wv_w * (vg + 1)], in_=vt[:, 0:wv_w]
                )

            # ---- steady state ----
            n_grp = (KCH + GRP - 1) // GRP
            for b in range(NBLK):
                acc = accp.tile([128, QB], fp32, tag="acc")
                q_rhs = qt[:, QB * b : QB * (b + 1)]
                for g in range(n_grp):
                    lo = g * GRP
                    hi = min(lo + GRP, KCH)
                    st = stp.tile([128, GRP * QB], fp32, tag="st")
                    ex = expp.tile([128, GRP * QB], f32r, tag="ex")
                    for i in range(hi - lo):
                        j = lo + i
                        nc.tensor.matmul(
                            st[:, QB * i : QB * (i + 1)],
                            lhsT=kt[:, 128 * j : 128 * (j + 1)],
                            rhs=q_rhs,
                            start=True,
                            stop=True,
                        )
                    w = QB * (hi - lo)
                    nc.scalar.activation(
                        out=ex[:, 0:w], in_=st[:, 0:w], func=Exp, scale=SCALE
                    )
                    for i in range(hi - lo):
                        j = lo + i
                        nc.tensor.matmul(
                            acc[0 : HV + 1, :],
                            lhsT=vp[:, (HV + 1) * j : (HV + 1) * (j + 1)],
                            rhs=ex[:, QB * i : QB * (i + 1)],
                            start=(j == 0),
                            stop=(j == KCH - 1),
                        )

                # ---- block epilogue ----
                accs = accsp.tile([HV + 1, QB], fp32, tag="accs")
                nc.vector.tensor_copy(out=accs[:], in_=acc[0 : HV + 1, :])
                tp = accp.tile([128, QB], fp32, tag="acc")
                for t in range(QB // 128):
                    nc.tensor.transpose(
                        tp[:, (HV + 1) * t : (HV + 1) * (t + 1)],
                        accs[:, 128 * t : 128 * (t + 1)],
                        ident[0 : HV + 1, 0 : HV + 1],
                    )
                rec = recp.tile([128, QB // 128], fp32, tag="rec")
                sums = tp[:, 0 : 4 * (HV + 1)].rearrange(
                    "p (t c) -> p t c", c=HV + 1
                )[:, :, HV]
                nc.vector.reciprocal(rec[:], sums)
                outt = outp.tile([128, 4 * HV], fp32, tag="outt")
                for t in range(QB // 128):
                    nc.vector.tensor_scalar_mul(
                        outt[:, HV * t : HV * (t + 1)],
                        tp[:, (HV + 1) * t : (HV + 1) * t + HV],
                        rec[:, t : t + 1],
                    )
                dview = out_d[QB * b : QB * (b + 1), :].rearrange(
                    "(t p) d -> p t d", p=128
                )
                with nc.allow_non_contiguous_dma(reason="64B-run output scatter"):
                    nc.sync.dma_start(
                        out=dview, in_=outt[:].rearrange("p (t d) -> p t d", d=HV)
                    )

    nc.compile()
    return nc


def _get_program():
    if "nc" not in _CACHE:
        _CACHE["nc"] = _build_program()
    return _CACHE["nc"]


def kernel(x, Wq, bq, Wk, bk, Wv, bv):
    global LAST_EXEC_NS
    from concourse.bass_utils import run_bass_kernel_spmd

    x = np.ascontiguousarray(np.asarray(x, dtype=np.float32))
    Wq = np.asarray(Wq, dtype=np.float32)
    bq = np.asarray(bq, dtype=np.float32)
    Wk = np.asarray(Wk, dtype=np.float32)
    bk = np.asarray(bk, dtype=np.float32)
    Wv = np.asarray(Wv, dtype=np.float32)
    bv = np.asarray(bv, dtype=np.float32)

    xfT = np.ascontiguousarray(x.T)                      # [64, 8192]
    xqT = [np.ascontiguousarray(x[i * NQ : (i + 1) * NQ].T) for i in range(2)]

    in_maps = []
    for c in range(8):
        h, qh = c // 2, c % 2
        wv_aug = np.zeros((F + 1, HV + 1), dtype=np.float32)
        wv_aug[0:F, 0:HV] = Wv[h]
        wv_aug[F, 0:HV] = bv[h]
        wv_aug[F, HV] = 1.0
        in_maps.append(
            {
                "xfT": xfT,
                "xqT": xqT[qh],
                "wq": np.ascontiguousarray(Wq[h]),
                "bq": np.ascontiguousarray(bq[h].reshape(KQ, 1)),
                "wk": np.ascontiguousarray(Wk[h]),
                "bk": np.ascontiguousarray(bk[h].reshape(KQ, 1)),
                "wv": wv_aug,
            }
        )

    nc = _get_program()
    trace = bool(os.environ.get("BASS_KERNEL_TRACE"))
    res = run_bass_kernel_spmd(nc, in_maps, list(range(8)), trace=trace)
    LAST_EXEC_NS = res.exec_time_ns

    out = np.empty((N, NH * HV), dtype=np.float32)
    for c in range(8):
        h, qh = c // 2, c % 2
        out[qh * NQ : (qh + 1) * NQ, h * HV : (h + 1) * HV] = res.results[c]["out"]
    return out
